# revision 86
# baseline (speedup 1.0000x reference)
"""Jamba sparse-MoE block on 8 Trainium2 NeuronCores.

Strategy
--------
- Routing (router matmul + softmax + top-2) is computed with jax on the host
  using the exact op sequence of the reference so expert selection matches
  bit-for-bit.
- Expert-parallel main path: core e runs the dense gate/up/silu/mul/down FFN
  of expert e over the first (up to) 2048 of its assigned tokens, gathered on
  the host.  Capping the main path at 2048 slots keeps all cores load-balanced
  regardless of routing skew.
- Tokens beyond 2048 per expert go to an overflow tail: 128-token
  single-expert blocks computed tensor-parallel (each core owns a 512-wide
  slice of the ffn dim) in single fp8 at the end of the same program; the
  host reduces the partial down-proj sums across cores.  Overflow is <1% of
  tokens, so the tail's ~3% per-token error is negligible in the norm.
- The main path's three matmuls run as fp8e4 (e4m3) DoubleRow matmuls (2x
  contraction per instruction at 0.5 cycles/row).  To stay within the 2e-2
  error budget each operand A is split into A_hi = fp8(A*S) and
  A_lo = fp8(A*S - A_hi) with a power-of-2 scale S that centers values in
  e4m3's normal range, and each product uses the 3-term compensation
      A @ B ~= A_hi@B_hi + A_lo@B_hi + A_hi@B_lo
  accumulated in fp32 PSUM (~6e-3 relative error end to end, measured).
  The descales fold into the activation scale (silu), the fused
  scalar_tensor_tensor multiply (up path), and the routing-weight scale
  (output), so no extra elementwise passes are needed.
- Phase A computes hid = silu(g)*u per (f-block, token-tile), splits it into
  fp8 hi/lo on ACT/DVE, and stages both to DRAM (per super-block/f-range
  tensors so phase B's reads can start as soon as their producers finish).
  Phase B streams hid back as the stationary operand and contracts over F
  against the SBUF-cached down weights.
- Scheduling notes that matter for the timeline: all tile pools live for the
  whole program (pool release inserts an all-engine drain barrier); DMAs are
  issued in dependency order with the startup-critical loads first; big
  transfers are split so none hogs the 16 shared DMA engines.
- Outputs are scatter-added back into the full [T, H] buffer on the host.
"""

import math
import numpy as np
from contextlib import ExitStack

B, S, H, F, E, TOP_K = 4, 2048, 1024, 4096, 8, 2
T = B * S
N_CORES = 8
P = 128
HC = H // P  # 8 h-chunks
FB = F // P  # 32 f-blocks
KP_A = HC // 2  # 4 DoubleRow k-pairs for the H contraction
FP_B = FB // 2  # 16 DoubleRow f-pairs for the F contraction

# power-of-2 scales that center each tensor in e4m3's normal range
SX = 2.0**4  # hidden states (~N(0,1))
SW = 2.0**9  # gate/up weights (~N(0, 0.02^2))
SH = 2.0**4  # hid = silu(g)*u
SD = 2.0**9  # down weights


def _token_tiles(C):
    assert C % 128 == 0 and C >= 512
    tiles = [512] * (C // 512)
    if C % 512:
        tiles.append(C % 512)
    return tiles


_PROGRAM_CACHE = {}


def _build_program(C, H_=H, F_=F, act="Silu", phases=3, NB=0):
    """SPMD program for one expert's FFN over C token slots, plus NB 128-token
    overflow blocks processed tensor-parallel (ffn split 8 ways) in single fp8.

    phases: bit0 = phase A, bit1 = phase B (debug/profiling aid).
    """
    key = (C, H_, F_, act, phases, NB)
    if key in _PROGRAM_CACHE:
        return _PROGRAM_CACHE[key]
    import concourse.bacc as bacc
    import concourse.mybir as mybir
    import concourse.tile as tile

    FB_ = F_ // P
    FP_ = FB_ // 2
    HC_ = H_ // P
    KP_ = HC_ // 2
    f32 = mybir.dt.float32
    f8 = mybir.dt.float8e4
    AF = mybir.ActivationFunctionType
    ALU = mybir.AluOpType
    DR = mybir.MatmulPerfMode.DoubleRow
    NT128 = C // P
    tiles = _token_tiles(C)

    nc = bacc.Bacc("TRN2", target_bir_lowering=False, debug=False, num_devices=N_CORES)

    # x is pre-chunked on host into 256-token chunks (padded up to C256) so
    # each chunk load is DMA-contiguous per partition
    C256 = ((C + 255) // 256) * 256
    NCH = C256 // 256
    xh_d = nc.dram_tensor("xh", [NCH, P, HC_, 256], f8, kind="ExternalInput")
    xl_d = nc.dram_tensor("xl", [NCH, P, HC_, 256], f8, kind="ExternalInput")
    gwh_d = nc.dram_tensor("gwh", [FB_, P, HC_, P], f8, kind="ExternalInput")
    gwl_d = nc.dram_tensor("gwl", [FB_, P, HC_, P], f8, kind="ExternalInput")
    uwh_d = nc.dram_tensor("uwh", [FB_, P, HC_, P], f8, kind="ExternalInput")
    uwl_d = nc.dram_tensor("uwl", [FB_, P, HC_, P], f8, kind="ExternalInput")
    dwh_d = nc.dram_tensor("dwh", [P, FB_, H_], f8, kind="ExternalInput")
    dwl_d = nc.dram_tensor("dwl", [P, FB_, H_], f8, kind="ExternalInput")
    wt_d = nc.dram_tensor("wt", [NT128, P], f32, kind="ExternalInput")
    y_d = nc.dram_tensor("y", [NT128, P, H_], f32, kind="ExternalOutput")
    # hid hi/lo staging, one DRAM tensor per (512-token super-block, 8-f-block
    # range) so each phase B read piece only depends on the matching phase A
    # writes
    sblocks = []
    t0 = 0
    while t0 < C:
        sblocks.append((t0, min(512, C - t0)))
        t0 += 512
    NR = FB_ // 8
    hh_ds = [
        [nc.dram_tensor(f"hh{i}_{r}", [P, 8, s[1]], f8) for r in range(NR)]
        for i, s in enumerate(sblocks)
    ]
    hl_ds = [
        [nc.dram_tensor(f"hl{i}_{r}", [P, 8, s[1]], f8) for r in range(NR)]
        for i, s in enumerate(sblocks)
    ]
    # overflow-tail tensors: NB blocks of 128 tokens, each a single expert's
    # overflow; every core computes a 512-wide slice of the ffn dim in bf16
    # and emits partial down-proj sums that the host reduces across cores
    FS_ = (F_ // N_CORES) // P  # f-slices of 128 per core
    if NB:
        xov_d = nc.dram_tensor("xov", [NB, P, HC_, P], f8, kind="ExternalInput")
        gov_d = nc.dram_tensor("gov", [NB, P, FS_, HC_, P], f8, kind="ExternalInput")
        uov_d = nc.dram_tensor("uov", [NB, P, FS_, HC_, P], f8, kind="ExternalInput")
        dov_d = nc.dram_tensor("dov", [NB, P, FS_, H_], f8, kind="ExternalInput")
        wtov_d = nc.dram_tensor("wtov", [P, NB], f32, kind="ExternalInput")
        yov_d = nc.dram_tensor("yov", [NB, P, H_], f32, kind="ExternalOutput")

    with tile.TileContext(nc) as tc:
        with ExitStack() as ctx:
            wtpool = ctx.enter_context(tc.tile_pool(name="wtp", bufs=1))
            dwhpool = ctx.enter_context(tc.tile_pool(name="dwhp", bufs=1))
            dwlpool = ctx.enter_context(tc.tile_pool(name="dwlp", bufs=1))

            wt_t = wtpool.tile([P, NT128], f32)
            # down-proj weights cached in SBUF for all of phase B; the DMAs
            # are issued mid-phase-A (below) to keep them off the critical
            # startup path
            dwh_t = dwhpool.tile([P, FB_, H_], f8)
            dwl_t = dwlpool.tile([P, FB_, H_], f8)

            # All pools live for the whole program: releasing a pool inserts
            # engine drains (a barrier), which would stall the PE at the
            # phase A -> phase B transition.
            # ---- Phase A: hid = silu(g) * u, split hi/lo, staged to DRAM ----
            with ExitStack() as actx:
              if phases & 1:
                psa = ctx.enter_context(tc.tile_pool(name="psa", bufs=4, space="PSUM"))
                xhpool = ctx.enter_context(tc.tile_pool(name="xhp", bufs=1))
                xlpool = ctx.enter_context(tc.tile_pool(name="xlp", bufs=1))
                ghpool = ctx.enter_context(tc.tile_pool(name="ghp", bufs=3))
                glpool = ctx.enter_context(tc.tile_pool(name="glp", bufs=3))
                uhpool = ctx.enter_context(tc.tile_pool(name="uhp", bufs=3))
                ulpool = ctx.enter_context(tc.tile_pool(name="ulp", bufs=3))
                sgpool = ctx.enter_context(tc.tile_pool(name="sgp", bufs=3))
                hspool = ctx.enter_context(tc.tile_pool(name="hsp", bufs=3))
                h8hpool = ctx.enter_context(tc.tile_pool(name="h8hp", bufs=3))
                h8lpool = ctx.enter_context(tc.tile_pool(name="h8lp", bufs=3))

                x8h_t = xhpool.tile([P, HC_, C256], f8)
                x8l_t = xlpool.tile([P, HC_, C256], f8)

                def load_xchunk(i, which=2):
                    if which != 1:
                        nc.sync.dma_start(
                            x8h_t[:, :, 256 * i : 256 * (i + 1)], xh_d.ap()[i]
                        )
                    if which != 0:
                        nc.sync.dma_start(
                            x8l_t[:, :, 256 * i : 256 * (i + 1)], xl_d.ap()[i]
                        )

                # Issue order matters twice over: DMAs transfer roughly in
                # issue order (earlier = sooner), and a tile read must come
                # AFTER its write in program order for the dependency tracker
                # to see it.  So: xh0 + fb0/fb1 weights first (first matmuls'
                # operands), then the rest of x before any fb0 matmul is
                # emitted.
                w_tiles = {}

                def load_weights(fb, eng=None):
                    eng = eng or nc.sync
                    gh_t = ghpool.tile([P, HC_, P], f8, name="gh")
                    eng.dma_start(gh_t[:], gwh_d.ap()[fb])
                    gl_t = glpool.tile([P, HC_, P], f8, name="gl")
                    eng.dma_start(gl_t[:], gwl_d.ap()[fb])
                    uh_t = uhpool.tile([P, HC_, P], f8, name="uh")
                    eng.dma_start(uh_t[:], uwh_d.ap()[fb])
                    ul_t = ulpool.tile([P, HC_, P], f8, name="ul")
                    eng.dma_start(ul_t[:], uwl_d.ap()[fb])
                    w_tiles[fb] = (gh_t, gl_t, uh_t, ul_t)

                load_xchunk(0, which=0)
                load_weights(0)
                # fb1's gate weights go via the idle Pool/SWDGE path so they
                # jump ahead of the x chunks in the DMA queue; its up weights
                # (needed ~2us later) stay on sync after chunk 1
                gh1 = ghpool.tile([P, HC_, P], f8, name="gh")
                nc.gpsimd.dma_start(gh1[:], gwh_d.ap()[1])
                gl1 = glpool.tile([P, HC_, P], f8, name="gl")
                nc.gpsimd.dma_start(gl1[:], gwl_d.ap()[1])
                load_xchunk(0, which=1)
                load_xchunk(1)
                uh1 = uhpool.tile([P, HC_, P], f8, name="uh")
                nc.sync.dma_start(uh1[:], uwh_d.ap()[1])
                ul1 = ulpool.tile([P, HC_, P], f8, name="ul")
                nc.sync.dma_start(ul1[:], uwl_d.ap()[1])
                w_tiles[1] = (gh1, gl1, uh1, ul1)
                for i in range(2, NCH):
                    load_xchunk(i)
                nc.sync.dma_start(wt_t[:], wt_d.ap().rearrange("n p -> p n"))

                def emit_fb_tile(wt4, fb, t0, nt):
                    gh_t, gl_t, uh_t, ul_t = wt4
                    ps_g = psa.tile([P, 512], f32, name="ps_g", bufs=3)[:, :nt]
                    ps_u = psa.tile([P, 512], f32, name="ps_u", bufs=3)[:, :nt]
                    for ps, w_hi, w_lo in (
                        (ps_g, gh_t, gl_t),
                        (ps_u, uh_t, ul_t),
                    ):
                        mm = 0
                        for w_t, x_t in (
                            (w_hi, x8h_t),
                            (w_hi, x8l_t),
                            (w_lo, x8h_t),
                        ):
                            for kp in range(KP_):
                                nc.tensor.matmul(
                                    ps,
                                    w_t[:, 2 * kp : 2 * kp + 2, :],
                                    x_t[:, 2 * kp : 2 * kp + 2, t0 : t0 + nt],
                                    start=(mm == 0),
                                    stop=(mm == 3 * KP_ - 1),
                                    perf_mode=DR,
                                )
                                mm += 1
                    sg = sgpool.tile([P, 512], f32, name="sg")[:, :nt]
                    nc.scalar.activation(
                        sg, ps_g, getattr(AF, act), scale=1.0 / (SX * SW)
                    )
                    hs = hspool.tile([P, 512], f32, name="hs")[:, :nt]
                    # hs = (ps_u * SH/(SX*SW)) * sg = hid * SH
                    nc.vector.scalar_tensor_tensor(
                        hs, ps_u, SH / (SX * SW), sg, ALU.mult, ALU.mult
                    )
                    h8h = h8hpool.tile([P, 512], f8, name="h8h")[:, :nt]
                    nc.scalar.activation(h8h, hs, AF.Copy)
                    h8l = h8lpool.tile([P, 512], f8, name="h8l")[:, :nt]
                    nc.vector.scalar_tensor_tensor(
                        h8l, hs, 1.0, h8h, ALU.mult, ALU.subtract
                    )
                    sb, off = t0 // 512, t0 % 512
                    nc.gpsimd.dma_start(
                        hh_ds[sb][fb // 8].ap()[:, fb % 8, off : off + nt], h8h
                    )
                    # the last f-block's lo writes go via Pool so SP's
                    # in-order queue drains early and phase B's hid reads
                    # (queued behind on SP) can issue before phase A ends
                    hl_eng = nc.gpsimd if fb == FB_ - 1 else nc.sync
                    hl_eng.dma_start(
                        hl_ds[sb][fb // 8].ap()[:, fb % 8, off : off + nt], h8l
                    )

                # f-blocks 0 and 1 interleave over token tiles: two f-blocks
                # consume each x chunk, so the matmuls stay ahead of the
                # incoming x stream.  The very first tile is split in two so
                # the first matmul only needs x chunk 0.
                emit_fb_tile(w_tiles[0], 0, 0, 256)
                emit_fb_tile(w_tiles[0], 0, 256, tiles[0] - 256)
                emit_fb_tile(w_tiles[1], 1, 0, tiles[0])
                t0 = tiles[0]
                for nt in tiles[1:]:
                    emit_fb_tile(w_tiles[0], 0, t0, nt)
                    emit_fb_tile(w_tiles[1], 1, t0, nt)
                    t0 += nt
                del w_tiles[0], w_tiles[1]

                for fb in range(2, FB_):
                    # down-proj weight cache streams in behind the phase A
                    # critical path, in small pieces so no single transfer
                    # hogs the DMA engines
                    if 8 <= fb < 16:
                        i = 4 * (fb - 8)
                        nc.sync.dma_start(
                            dwh_t[:, i : i + 4, :], dwh_d.ap()[:, i : i + 4, :]
                        )
                    elif 16 <= fb < 24:
                        i = 4 * (fb - 16)
                        nc.sync.dma_start(
                            dwl_t[:, i : i + 4, :], dwl_d.ap()[:, i : i + 4, :]
                        )
                    load_weights(fb)
                    wt4 = w_tiles.pop(fb)
                    t0 = 0
                    for nt in tiles:
                        emit_fb_tile(wt4, fb, t0, nt)
                        t0 += nt

            # ---- Phase B: y[t, :] = wt[t] * (hid[:, t].T @ dw.T) ----
            hhpool = ctx.enter_context(tc.tile_pool(name="hhp", bufs=2))
            hlpool = ctx.enter_context(tc.tile_pool(name="hlp", bufs=1))
            ypool = ctx.enter_context(tc.tile_pool(name="yp", bufs=3))

            for sb, (t0, snt) in enumerate(sblocks if (phases & 2) else []):
                hh_t = hhpool.tile([P, FB_, 512], f8, name="hh_t")[:, :, :snt]
                hl_t = hlpool.tile([P, FB_, 512], f8, name="hl_t")[:, :, :snt]
                for r in range(NR):
                    nc.sync.dma_start(hh_t[:, 8 * r : 8 * r + 8, :], hh_ds[sb][r].ap())
                    nc.sync.dma_start(hl_t[:, 8 * r : 8 * r + 8, :], hl_ds[sb][r].ap())
                for ts in range(snt // P):
                    tt = t0 // P + ts
                    for hi in range(H_ // 512):
                        ps_y = psa.tile([P, 512], f32, name="ps_y", bufs=2)
                        mm = 0
                        for h_t, d_t in (
                            (hh_t, dwh_t),
                            (hl_t, dwh_t),
                            (hh_t, dwl_t),
                        ):
                            for fp in range(FP_):
                                nc.tensor.matmul(
                                    ps_y,
                                    h_t[:, 2 * fp : 2 * fp + 2, ts * P : (ts + 1) * P],
                                    d_t[:, 2 * fp : 2 * fp + 2, hi * 512 : (hi + 1) * 512],
                                    start=(mm == 0),
                                    stop=(mm == 3 * FP_ - 1),
                                    perf_mode=DR,
                                )
                                mm += 1
                        y_sb = ypool.tile([P, 512], f32, name="y_sb")
                        nc.scalar.activation(
                            y_sb, ps_y, AF.Copy, scale=wt_t[:, tt : tt + 1]
                        )
                        nc.scalar.dma_start(
                            y_d.ap()[tt][:, hi * 512 : (hi + 1) * 512], y_sb
                        )

            # ---- Overflow tail: NB 128-token blocks, ffn sliced 8 ways ----
            # Single-fp8 (no hi/lo, no DoubleRow) with the same power-of-2
            # scales as the main path: these are <1% of tokens, so the ~3%
            # per-token error is negligible in the norm.
            if NB and (phases & 2):
                xovpool = ctx.enter_context(tc.tile_pool(name="xovp", bufs=2))
                govpool = ctx.enter_context(tc.tile_pool(name="govp", bufs=2))
                uovpool = ctx.enter_context(tc.tile_pool(name="uovp", bufs=2))
                dovpool = ctx.enter_context(tc.tile_pool(name="dovp", bufs=2))

                wtov_t = wtpool.tile([P, NB], f32, name="wtov")
                nc.sync.dma_start(wtov_t[:], wtov_d.ap())
                h8s = {}

                def emit_ov_down(b):
                    h8, dt = h8s.pop(b)
                    for hi in range(H_ // 512):
                        ps_y = psa.tile([P, 512], f32, name="ps_y", bufs=2)
                        for fs in range(FS_):
                            nc.tensor.matmul(
                                ps_y,
                                h8[:, fs * P : (fs + 1) * P],
                                dt[:, fs, hi * 512 : (hi + 1) * 512],
                                start=(fs == 0),
                                stop=(fs == FS_ - 1),
                            )
                        yo = ypool.tile([P, 512], f32, name="y_sb")
                        nc.scalar.activation(
                            yo, ps_y, AF.Copy, scale=wtov_t[:, b : b + 1]
                        )
                        nc.scalar.dma_start(
                            yov_d.ap()[b][:, hi * 512 : (hi + 1) * 512], yo
                        )

                for b in range(NB):
                    xb = xovpool.tile([P, HC_, P], f8, name="xov")
                    nc.sync.dma_start(xb[:], xov_d.ap()[b])
                    gt = govpool.tile([P, FS_, HC_, P], f8, name="gov")
                    nc.sync.dma_start(gt[:], gov_d.ap()[b])
                    ut = uovpool.tile([P, FS_, HC_, P], f8, name="uov")
                    nc.sync.dma_start(ut[:], uov_d.ap()[b])
                    dt = dovpool.tile([P, FS_, H_], f8, name="dov")
                    nc.sync.dma_start(dt[:], dov_d.ap()[b])
                    # one wide psum tile per block; each 128-col slice is its
                    # own accumulation group (one per f-slice)
                    ps_g = psa.tile([P, 512], f32, name="ps_g", bufs=3)
                    ps_u = psa.tile([P, 512], f32, name="ps_u", bufs=3)
                    for fs in range(FS_):
                        for ps, w_t in ((ps_g, gt), (ps_u, ut)):
                            for s in range(HC_):
                                nc.tensor.matmul(
                                    ps[:, fs * P : (fs + 1) * P],
                                    w_t[:, fs, s, :],
                                    xb[:, s, :],
                                    start=(s == 0),
                                    stop=(s == HC_ - 1),
                                )
                    sg = sgpool.tile([P, 512], f32, name="sg")
                    nc.scalar.activation(
                        sg, ps_g, getattr(AF, act), scale=1.0 / (SX * SW)
                    )
                    h8 = h8hpool.tile([P, 512], f8, name="h8h")
                    nc.vector.scalar_tensor_tensor(
                        h8, ps_u, SH / (SX * SW), sg, ALU.mult, ALU.mult
                    )
                    h8s[b] = (h8, dt)
                    # pipeline: the previous block's down-proj fills the PE
                    # while this block's silu/cast chain drains
                    if b >= 1:
                        emit_ov_down(b - 1)
                emit_ov_down(NB - 1)
    nc.compile()
    _PROGRAM_CACHE[key] = nc
    return nc


def _routing(hidden_states, router_w):
    """Replicate the reference's routing ops exactly (same jax ops, default
    platform) so top-2 selection matches bit-for-bit."""
    import jax
    import jax.numpy as jnp

    x = jnp.asarray(hidden_states).reshape(-1, H)
    router_logits = x @ jnp.asarray(router_w).T
    routing_weights = jax.nn.softmax(router_logits.astype(jnp.float32), axis=-1)
    top_k_weights, top_k_index = jax.lax.top_k(routing_weights, TOP_K)
    return np.asarray(top_k_index), np.asarray(top_k_weights, dtype=np.float32)


def _f8(a):
    import ml_dtypes

    return np.ascontiguousarray(a.astype(ml_dtypes.float8_e4m3))


def _split8(a, scale):
    """Return (hi, lo) e4m3 arrays such that hi + lo ~= a * scale."""
    a = a * np.float32(scale)
    hi = _f8(a)
    lo = _f8(a - hi.astype(np.float32))
    return hi, lo


_WEIGHT_CACHE = {}


def _expert_weights(e, gate_w, up_w, down_w):
    """Per-expert fp8 hi/lo weight splits in device layout (cached)."""
    key = e
    ent = _WEIGHT_CACHE.get(key)
    if ent is not None and ent[0] is gate_w and ent[1] is up_w and ent[2] is down_w:
        return ent[3]
    gw = gate_w[e].reshape(FB, P, HC, P).transpose(0, 3, 2, 1)  # [fb, p, s, j]
    uw = up_w[e].reshape(FB, P, HC, P).transpose(0, 3, 2, 1)
    dw = down_w[e].T.reshape(FB, P, H).transpose(1, 0, 2)  # [p, fb, h]
    gwh, gwl = _split8(gw, SW)
    uwh, uwl = _split8(uw, SW)
    dwh, dwl = _split8(dw, SD)
    out = {"gwh": gwh, "gwl": gwl, "uwh": uwh, "uwl": uwl, "dwh": dwh, "dwl": dwl}
    _WEIGHT_CACHE[key] = (gate_w, up_w, down_w, out)
    return out


def kernel(hidden_states, router_w, gate_w, up_w, down_w):
    from concourse.bass_utils import run_bass_kernel_spmd

    hidden_states = np.asarray(hidden_states, dtype=np.float32)
    router_w = np.asarray(router_w, dtype=np.float32)
    gate_w = np.asarray(gate_w, dtype=np.float32)
    up_w = np.asarray(up_w, dtype=np.float32)
    down_w = np.asarray(down_w, dtype=np.float32)

    tki, tkw = _routing(hidden_states, router_w)
    xf = hidden_states.reshape(T, H)

    CAP = 2048  # main-path token slots per core; the rest goes to the tail
    idx_list, w_list, ov_blocks = [], [], []
    for e in range(E):
        sel = tki == e  # [T, 2]
        tok = sel.any(axis=1)
        idx = np.nonzero(tok)[0]
        w = np.where(sel[:, 0], tkw[:, 0], tkw[:, 1])[idx].astype(np.float32)
        idx_list.append(idx[:CAP])
        w_list.append(w[:CAP])
        for off in range(CAP, len(idx), P):
            ov_blocks.append((e, idx[off : off + P], w[off : off + P]))

    max_ne = max(len(i) for i in idx_list)
    C = max(512, int(math.ceil(max_ne / 128.0)) * 128)
    NT128 = C // P
    NB = max(1, len(ov_blocks))

    import ml_dtypes

    f8 = ml_dtypes.float8_e4m3
    nc = _build_program(C, NB=NB)

    # tail inputs shared by all cores: tokens and combine weights per block
    FS = (F // N_CORES) // P
    xov = np.zeros((NB, P, HC, P), f8)
    wtov = np.zeros((P, NB), np.float32)
    for b, (e, bidx, bw) in enumerate(ov_blocks):
        nb = len(bidx)
        xb = np.zeros((P, H), np.float32)
        xb[:nb] = xf[bidx]
        xov[b] = (xb.T.reshape(HC, P, P).transpose(1, 0, 2) * np.float32(SX)).astype(f8)
        wtov[:nb, b] = bw / np.float32(SH * SD)

    in_maps = []
    for e in range(E):
        idx, w = idx_list[e], w_list[e]
        ne = len(idx)
        xg = np.zeros((C, H), np.float32)
        xg[:ne] = xf[idx]
        wp = np.zeros((C,), np.float32)
        wp[:ne] = w / np.float32(SH * SD)  # fold the hid/down descale in
        # chunked device layout [chunk, p, s, 256] so each chunk load is
        # contiguous per partition; x is padded up to a 256 multiple
        C256 = ((C + 255) // 256) * 256
        xgp = xg if C256 == C else np.pad(xg, ((0, C256 - C), (0, 0)))
        xdev = np.ascontiguousarray(
            xgp.T.reshape(HC, P, C256 // 256, 256).transpose(2, 1, 0, 3)
        )
        xh, xl = _split8(xdev, SX)
        m = {"xh": xh, "xl": xl, "wt": np.ascontiguousarray(wp.reshape(NT128, P))}
        m.update(_expert_weights(e, gate_w, up_w, down_w))
        # per-core tail weight slices: this core owns f rows [c*512, (c+1)*512)
        c = e
        gov = np.zeros((NB, P, FS, HC, P), f8)
        uov = np.zeros((NB, P, FS, HC, P), f8)
        dov = np.zeros((NB, P, FS, H), f8)
        for b, (eb, _bi, _bw) in enumerate(ov_blocks):
            for fs in range(FS):
                f0 = c * (F // N_CORES) + fs * P
                gov[b, :, fs] = (
                    gate_w[eb][f0 : f0 + P].T.reshape(HC, P, P).transpose(1, 0, 2)
                    * np.float32(SW)
                ).astype(f8)
                uov[b, :, fs] = (
                    up_w[eb][f0 : f0 + P].T.reshape(HC, P, P).transpose(1, 0, 2)
                    * np.float32(SW)
                ).astype(f8)
                dov[b, :, fs] = (down_w[eb][:, f0 : f0 + P].T * np.float32(SD)).astype(f8)
        m.update(
            {
                "xov": xov,
                "gov": gov,
                "uov": uov,
                "dov": dov,
                "wtov": np.ascontiguousarray(wtov),
            }
        )
        in_maps.append(m)

    res = run_bass_kernel_spmd(nc, in_maps, core_ids=list(range(N_CORES)))

    out = np.zeros((T, H), np.float32)
    for e in range(E):
        idx = idx_list[e]
        y = res.results[e]["y"].reshape(C, H)
        out[idx] += y[: len(idx)]
    yov = np.zeros((NB, P, H), np.float64)
    for c in range(N_CORES):
        yov += res.results[c]["yov"].astype(np.float64)
    for b, (e, bidx, bw) in enumerate(ov_blocks):
        out[bidx] += yov[b][: len(bidx)].astype(np.float32)
    return out.reshape(B, S, H)
